# revision 20
# baseline (speedup 1.0000x reference)
"""Trainium2 Bass kernel for nn_AttentionModule (B=2, L=2048, D=1024, H=16).

Sharding: 8 cores = 2 batches x 4 head-groups (4 heads each).

v3 schedule ("e-stationary PV"):
  - QKV: qT/kT = W x^T (features on partitions, head pair per j-block),
    v natural [lk, 65] with a ones column (col 64) per head.
  - Attention runs in 8 slots (4 heads x 2 lq-windows of 1024). Per slot,
    per kc: scoresT[k,q] into a [128,1024] PSUM tile, exp on ACT (the
    cadence-setting engine at ~17.8us/slot), bias multiply on DVE (bf16
    2x mode) into e_sb.
  - PV is "e-stationary": for each lq-chunk of 128, accumulate
    e[lk,lq].T @ v65[lk] over 16 lk chunks into a [128,65] PSUM tile.
    Full 128x128 PE utilization (vs 65/128 for v-stationary); the
    denominator lands in column 64 as a per-partition scalar: reciprocal
    approx + tensor_scalar_mul normalize with no partition broadcast.
    attn_nat [128,64] is PE-transposed into attnT pairs [128, L] (two
    heads stacked) for a K=128 output projection (64 matmuls, tail).
  - PV/norm/transpose of slot s runs inside slot s+1's scores/exp
    stream; v matmuls fill slot 0, qk[j1] fills slots 1-2, so the PE
    never idles long enough to drop out of its top p-state.
  - Biases: bq/bk applied on-chip in the QKV psum eviction; bv/bo are
    folded into a host-side constant (softmax weights sum to 1, so
    attn(v + bv) = attn(v) + bv and the output shift is Wo @ bv + bo).

Host sums the 4 head-group partials per batch (fp16), transposes, adds
bo + Wo @ bv.
"""

import os
import numpy as np
import ml_dtypes

B, L, D, H = 2, 2048, 1024, 16
HD = D // H              # 64
SCALE = 1.0 / (HD ** 0.5)
NCORES = 8
NG = 4                   # head groups
HPG = H // NG            # 4 heads per core
DG = HPG * HD            # 256 features per core
BF16 = ml_dtypes.bfloat16

FW = 512                 # matmul max free dim
EW = 1024                # exp window (2 psum banks)
NW = L // EW             # 2 lq windows per head
NQW = L // FW            # 4
NKC = L // 128           # 16
NFC = D // 128           # 8
NOB = D // 128           # 8 output blocks
AW = HD + 1              # v columns per head (64 feats + ones)
NLQ = EW // 128          # 8 lq chunks per window
NSLOT = HPG * NW         # 8

_prog_cache = {}

LAST_RESULTS = None      # BassKernelResults of the most recent run


def _build_program():
    from contextlib import ExitStack
    import concourse.tile as tile
    from concourse import bacc, mybir

    dt = mybir.dt
    AF = mybir.ActivationFunctionType

    nc = bacc.Bacc(
        "TRN2", target_bir_lowering=False, debug=False, num_devices=NCORES
    )

    xT_d = nc.dram_tensor("xT", (D, L), dt.bfloat16, kind="ExternalInput").ap()
    wqT_d = nc.dram_tensor("wqT", (D, DG), dt.bfloat16, kind="ExternalInput").ap()
    wkT_d = nc.dram_tensor("wkT", (D, DG), dt.bfloat16, kind="ExternalInput").ap()
    wvT_d = nc.dram_tensor("wvT", (D, DG), dt.bfloat16, kind="ExternalInput").ap()
    # wo pairs: [128 rows = 2 heads x 64 feats, pair, D]
    woP_d = nc.dram_tensor("woP", (128, 2, D), dt.bfloat16, kind="ExternalInput").ap()
    bq_d = nc.dram_tensor("bq", (128, 2), dt.float32, kind="ExternalInput").ap()
    bk_d = nc.dram_tensor("bk", (128, 2), dt.float32, kind="ExternalInput").ap()
    id_d = nc.dram_tensor("ident", (128, 128), dt.bfloat16, kind="ExternalInput").ap()
    expbT_d = nc.dram_tensor(
        "expbT", (HPG, L, L), dt.bfloat16, kind="ExternalInput"
    ).ap()
    out_d = nc.dram_tensor("out", (D, L), dt.float16, kind="ExternalOutput").ap()

    with tile.TileContext(nc) as tc, ExitStack() as ctx:
        consts = ctx.enter_context(tc.tile_pool(name="consts", bufs=1))

        # q/k weights + biases first so QKV can start as early as possible
        wq_sb = consts.tile([128, NFC, DG], dt.bfloat16)
        nc.sync.dma_start(out=wq_sb, in_=wqT_d.rearrange("(c p) m -> p c m", p=128))
        wk_sb = consts.tile([128, NFC, DG], dt.bfloat16)
        nc.sync.dma_start(out=wk_sb, in_=wkT_d.rearrange("(c p) m -> p c m", p=128))
        bq_sb = consts.tile([128, 2], dt.float32)
        nc.sync.dma_start(out=bq_sb, in_=bq_d)
        bk_sb = consts.tile([128, 2], dt.float32)
        nc.sync.dma_start(out=bk_sb, in_=bk_d)
        id_sb = consts.tile([128, 128], dt.bfloat16)
        nc.sync.dma_start(out=id_sb, in_=id_d)

        # x chunks on the gpsimd DMA queue so they overlap the weight loads
        x_sb = consts.tile([128, NFC, L], dt.bfloat16)
        for fc in range(NFC):
            nc.gpsimd.dma_start(
                out=x_sb[:, fc, :], in_=xT_d[fc * 128 : (fc + 1) * 128, :]
            )

        wv_sb = consts.tile([128, NFC, DG], dt.bfloat16)
        nc.sync.dma_start(out=wv_sb, in_=wvT_d.rearrange("(c p) m -> p c m", p=128))
        wo_sb = consts.tile([128, 2, D], dt.bfloat16)
        nc.gpsimd.dma_start(out=wo_sb, in_=woP_d)

        qT_sb = consts.tile([128, 2, L], dt.bfloat16)
        kT_sb = consts.tile([128, 2, L], dt.bfloat16)
        v_sb = consts.tile([128, NKC, HPG, AW], dt.bfloat16)
        # e: [parity, kc, lq-in-window]
        e_sb = consts.tile([128, 2, NKC, EW], dt.bfloat16)
        # attnT pairs: rows = (h%2)*64 + feat, [pair, L]
        attnP_sb = consts.tile([128, 2, L], dt.bfloat16)
        attn_nat = consts.tile([128, NLQ, HD], dt.bfloat16)
        rec_sb = consts.tile([128, NLQ, 1], dt.float32)

        # ones column 64 of each v65 block: softmax denominator accumulator
        nc.vector.memset(v_sb[:, :, :, HD : HD + 1], 1.0)

        eb_p = ctx.enter_context(tc.tile_pool(name="eb_p", bufs=4))

        # ---------------- PSUM pools (strict LIFO stack) ----------------
        # prefix/s0: qk(1) + sc(6) + vv(1) = 8 banks
        # s1-s7:     qk(1) + sc(6) + pv(1) = 8            (vv closed)
        # tail:      qk(1) + op(3) = 4                    (pv/sc closed)
        qk_cm = tc.tile_pool(name="qk_ps", bufs=1, space="PSUM")
        qk_ps = qk_cm.__enter__()
        sc_cm = tc.tile_pool(name="sc_ps", bufs=3, space="PSUM")
        sc_ps = sc_cm.__enter__()
        vv_cm = tc.tile_pool(name="vv_ps", bufs=1, space="PSUM")
        vv_ps = vv_cm.__enter__()

        # ---------------- building blocks ----------------
        def qk_lw(w_sb, b_sb, dst, j, lw, engine):
            """One [128,512] column block of q or k projection + evict."""
            ps = qk_ps.tile([128, FW], dt.float32, tag="qk", name="qk_ps_t")
            for fc in range(NFC):
                nc.tensor.matmul(
                    ps,
                    w_sb[:, fc, j * 128 : (j + 1) * 128],
                    x_sb[:, fc, lw * FW : (lw + 1) * FW],
                    start=(fc == 0),
                    stop=(fc == NFC - 1),
                )
                yield
            if engine == "act":
                nc.scalar.activation(
                    out=dst[:, j, lw * FW : (lw + 1) * FW],
                    in_=ps,
                    func=AF.Identity,
                    bias=b_sb[:, j : j + 1],
                    scale=1.0,
                )
            else:
                nc.vector.tensor_scalar_add(
                    dst[:, j, lw * FW : (lw + 1) * FW], ps, b_sb[:, j : j + 1]
                )

        def v_block(lb, engine="dve"):
            """v natural for one 128-row chunk of L, all 4 heads."""
            ps = vv_ps.tile([128, DG], dt.float32, tag="v", name="v_ps_t")
            for fc in range(NFC):
                nc.tensor.matmul(
                    ps,
                    x_sb[:, fc, lb * 128 : (lb + 1) * 128],
                    wv_sb[:, fc, :],
                    start=(fc == 0),
                    stop=(fc == NFC - 1),
                )
                yield
            out_ap = v_sb[:, lb, :, 0:HD]
            in_ap = ps.rearrange("p (h d) -> p h d", h=HPG)
            if engine == "act":
                nc.scalar.copy(out=out_ap, in_=in_ap)
            else:
                nc.vector.tensor_copy(out=out_ap, in_=in_ap)

        def slot_hw(s):
            return s // NW, s % NW

        def a_work(s, kc):
            """scores + exp + mul for slot s, chunk kc."""
            h, w = slot_hw(s)
            hp = (h % 2) * HD
            jb = h // 2
            eb_t = eb_p.tile([128, EW], dt.bfloat16, tag="eb", name="eb_t")
            nc.sync.dma_start(
                out=eb_t,
                in_=expbT_d[h, kc * 128 : (kc + 1) * 128, w * EW : (w + 1) * EW],
            )
            sc = sc_ps.tile([128, EW], dt.float32, tag="sc", name="sc")
            for q2 in range(2):
                nc.tensor.matmul(
                    sc[:, q2 * FW : (q2 + 1) * FW],
                    kT_sb[hp : hp + HD, jb, kc * 128 : (kc + 1) * 128],
                    qT_sb[hp : hp + HD, jb, w * EW + q2 * FW : w * EW + (q2 + 1) * FW],
                    start=True,
                    stop=True,
                )
            e_dst = e_sb[:, s % 2, kc, :]
            nc.scalar.activation(out=e_dst, in_=sc, func=AF.Exp, bias=0.0, scale=SCALE)
            nc.vector.tensor_mul(e_dst, e_dst, eb_t)

        def b_work(s, lq):
            """PV + norm + transpose for slot s, lq chunk (0..7)."""
            h, w = slot_hw(s)
            par = s % 2
            pv = pv_ps.tile([128, AW], dt.float32, tag="pv", name="pv")
            for lk in range(NKC):
                nc.tensor.matmul(
                    pv,
                    e_sb[:, par, lk, lq * 128 : (lq + 1) * 128],
                    v_sb[:, lk, h, :],
                    start=(lk == 0),
                    stop=(lk == NKC - 1),
                )
            nc.vector.reciprocal_approx_fast(
                out=rec_sb[:, lq, :], in_=pv[:, HD : HD + 1]
            )
            nc.vector.tensor_scalar_mul(
                attn_nat[:, lq, :], pv[:, 0:HD], rec_sb[:, lq, :]
            )
            # SBUF->SBUF XBAR transpose into the attnT pair layout
            nc.sync.dma_start_transpose(
                out=attnP_sb[
                    (h % 2) * HD : (h % 2) * HD + HD,
                    h // 2,
                    w * EW + lq * 128 : w * EW + (lq + 1) * 128,
                ],
                in_=attn_nat[:, lq, :],
            )

        def drain(gen):
            for _ in gen:
                pass

        # ---------------- prefix: qk j0 + first 4 v blocks ----------------
        for lw in range(NQW):
            drain(qk_lw(wq_sb, bq_sb, qT_sb, 0, lw, "act"))
        for lw in range(NQW):
            drain(qk_lw(wk_sb, bk_sb, kT_sb, 0, lw, "act"))
        for lb in range(4):
            drain(v_block(lb, engine="act"))

        # ---------------- slot 0: A-work + 12 v blocks (6 mm/kc) ----------
        vfill = (mm for lb in range(4, NKC) for mm in v_block(lb))
        for kc in range(NKC):
            a_work(0, kc)
            for _ in range(6):
                next(vfill, None)
        drain(vfill)
        vv_cm.__exit__(None, None, None)  # release v psum banks

        pv_cm = tc.tile_pool(name="pv_ps", bufs=1, space="PSUM")
        pv_ps = pv_cm.__enter__()

        # ---------------- slots 1..7 ----------------
        # qk j1 fillers: q in slot 1 (DVE evict), k in slot 2 (Pool evict)
        qfill = (mm for lw in range(NQW)
                 for mm in qk_lw(wq_sb, bq_sb, qT_sb, 1, lw, "dve"))
        kfill = (mm for lw in range(NQW)
                 for mm in qk_lw(wk_sb, bk_sb, kT_sb, 1, lw, "dve"))
        for s in range(1, NSLOT):
            for kc in range(NKC):
                a_work(s, kc)
                if s == 1:
                    next(qfill, None)
                    next(qfill, None)
                    if kc >= 8:
                        b_work(s - 1, kc - 8)
                elif s == 2:
                    next(kfill, None)
                    next(kfill, None)
                    if kc % 2 == 0:
                        b_work(s - 1, kc // 2)
                else:
                    if kc % 2 == 0:
                        b_work(s - 1, kc // 2)
            if s == 1:
                drain(qfill)
            elif s == 2:
                drain(kfill)

        for lq in range(NLQ):
            b_work(NSLOT - 1, lq)
        pv_cm.__exit__(None, None, None)
        sc_cm.__exit__(None, None, None)

        # ---------------- output projection (pairs, K=128) ----------------
        with (
            tc.tile_pool(name="op_ps", bufs=3, space="PSUM") as op_ps,
            tc.tile_pool(name="ob_p", bufs=3) as ob_p,
        ):
            for ob in range(NOB):
                for qw in range(NQW):
                    ps = op_ps.tile([128, FW], dt.float32, tag="op", name="op")
                    for j in range(2):
                        nc.tensor.matmul(
                            ps,
                            wo_sb[:, j, ob * 128 : (ob + 1) * 128],
                            attnP_sb[:, j, qw * FW : (qw + 1) * FW],
                            start=(j == 0),
                            stop=(j == 1),
                        )
                    o_t = ob_p.tile([128, FW], dt.float16, tag="ot", name="o_t")
                    if (ob * NQW + qw) % 2 == 0:
                        nc.scalar.copy(out=o_t, in_=ps)
                    else:
                        nc.vector.tensor_copy(out=o_t, in_=ps)
                    nc.gpsimd.dma_start(
                        out=out_d[
                            ob * 128 : (ob + 1) * 128, qw * FW : (qw + 1) * FW
                        ],
                        in_=o_t,
                    )
        qk_cm.__exit__(None, None, None)

    nc.compile()
    return nc


def _get_program():
    key = ("v3",)
    if key not in _prog_cache:
        _prog_cache[key] = _build_program()
    return _prog_cache[key]


def make_in_maps(inputs):
    x = np.asarray(inputs["x"], np.float32)
    biases = np.asarray(inputs["attn_biases"], np.float32)
    mask = np.asarray(inputs["attn_mask"], np.float32)
    Wq = np.asarray(inputs["Wq"], np.float32)
    Wk = np.asarray(inputs["Wk"], np.float32)
    Wv = np.asarray(inputs["Wv"], np.float32)
    Wo = np.asarray(inputs["Wo"], np.float32)
    bq = np.asarray(inputs["bq"], np.float32)
    bk = np.asarray(inputs["bk"], np.float32)

    ident = np.eye(128, dtype=BF16)
    mask_any = bool(np.any(mask))
    in_maps = []
    for c in range(NCORES):
        b, g = divmod(c, NG)
        rows = slice(g * DG, (g + 1) * DG)
        logits = biases[b, g * HPG : (g + 1) * HPG]  # (HPG, Lq, Lk)
        if mask_any:
            logits = logits + mask[b, 0][None]
        expbT = np.exp(logits).transpose(0, 2, 1).astype(BF16)  # (HPG, Lk, Lq)
        expbT = np.ascontiguousarray(expbT)
        # wo pairs: [128 rows (2 heads x 64 feats), pair, D]
        woP = np.empty((128, 2, D), np.float32)
        for j in range(2):
            cols = slice(g * DG + 2 * j * HD, g * DG + 2 * (j + 1) * HD)
            woP[:, j, :] = Wo[:, cols].T
        in_maps.append(
            {
                "xT": np.ascontiguousarray(x[b].T).astype(BF16),
                "wqT": np.ascontiguousarray(Wq[rows, :].T).astype(BF16),
                "wkT": np.ascontiguousarray(Wk[rows, :].T).astype(BF16),
                "wvT": np.ascontiguousarray(Wv[rows, :].T).astype(BF16),
                "woP": woP.astype(BF16),
                "bq": np.ascontiguousarray(bq[rows].reshape(2, 128).T),
                "bk": np.ascontiguousarray(bk[rows].reshape(2, 128).T),
                "ident": ident,
                "expbT": expbT,
            }
        )
    return in_maps


def kernel(**inputs):
    global LAST_RESULTS
    from concourse.bass_utils import run_bass_kernel_spmd

    bo = np.asarray(inputs["bo"], np.float32)
    bv = np.asarray(inputs["bv"], np.float32)
    Wo = np.asarray(inputs["Wo"], np.float32)
    out_shift = bo + Wo @ bv  # attn(v + bv) = attn(v) + bv; shift = Wo @ bv

    nc = _get_program()
    in_maps = make_in_maps(inputs)

    trace = bool(int(os.environ.get("KERNEL_TRACE", "0")))
    LAST_RESULTS = run_bass_kernel_spmd(
        nc, in_maps, list(range(NCORES)), trace=trace
    )
    res = LAST_RESULTS.results

    out = np.empty((B, L, D), np.float32)
    for b in range(B):
        acc = res[b * NG]["out"].astype(np.float32)
        for g in range(1, NG):
            acc += res[b * NG + g]["out"].astype(np.float32)
        out[b] = acc.T + out_shift
    return out
